# revision 36
# baseline (speedup 1.0000x reference)
"""Multi-head causal attention (B=2, T=2048, D=1024, H=16) on 8 trn2 NeuronCores.

Sharding: data-parallel over batch (2) x tensor-parallel over heads (4 groups of
4 heads). Core c handles batch c//4, head group c%4. Each core computes its
heads' attention and a partial output projection; the host sums the 4 partials
per batch and adds b_out.

v2: all-bf16 matmul operands (host-side casts), software-pipelined attention
inner loop with QKV/proj matmuls pumped as PE fillers, slim normalization path,
gpsimd offload for mask adds / normalize muls / PSUM evictions, bf16 output.
"""

import collections
import sys

sys.path.insert(0, "/opt/trn_rl_repo")

import ml_dtypes
import numpy as np

import concourse.bass as bass
import concourse.tile as tile
from concourse import bacc, mybir
from concourse.bass_utils import run_bass_kernel_spmd

F32 = mybir.dt.float32
BF16 = mybir.dt.bfloat16
BF = ml_dtypes.bfloat16

B, T, D, H = 2, 2048, 1024, 16
DH = D // H            # 64
HG = 4                 # heads per core
GCOLS = HG * DH        # 256 columns of q/k/v per core
NKT = T // 128         # 16 k-tiles of 128
NQC = T // 512         # 4 q-chunks of 512
NDT = D // 128         # 8 d-tiles of 128 (contraction)

_CACHED = {}


def _build():
    nc = bacc.Bacc("TRN2", target_bir_lowering=False, debug=False, num_devices=8)

    xT = nc.dram_tensor("xT", [D, T], BF16, kind="ExternalInput").ap()
    wqkv = nc.dram_tensor("wqkv", [D, 3 * GCOLS], BF16, kind="ExternalInput").ap()
    bqp = nc.dram_tensor("bqp", [128, 4], F32, kind="ExternalInput").ap()
    bv = nc.dram_tensor("bv", [1, GCOLS], BF16, kind="ExternalInput").ap()
    wout = nc.dram_tensor("wout", [GCOLS, D], BF16, kind="ExternalInput").ap()
    # additive causal mask for diagonal blocks, duplicated for both halves:
    # [128 k, 2*128 q] with 0.0 where k<=q else -1e30
    mask2 = nc.dram_tensor("mask2", [128, 256], F32, kind="ExternalInput").ap()
    # consts row 0: ones
    consts = nc.dram_tensor("consts", [3, 128], BF16, kind="ExternalInput").ap()
    out = nc.dram_tensor("out", [T, D], BF16, kind="ExternalOutput").ap()

    Exp = mybir.ActivationFunctionType.Exp
    Ident = mybir.ActivationFunctionType.Identity

    with tile.TileContext(nc) as tc:
        with tc.tile_pool(name="const", bufs=1) as const, \
             tc.tile_pool(name="ps_qkv", bufs=2, space=bass.MemorySpace.PSUM) as ps_qkv, \
             tc.tile_pool(name="ps_s", bufs=2, space=bass.MemorySpace.PSUM) as ps_s, \
             tc.tile_pool(name="ps_o", bufs=1, space=bass.MemorySpace.PSUM) as ps_o, \
             tc.tile_pool(name="ppool", bufs=8) as ppool, \
             tc.tile_pool(name="rpool", bufs=2) as rpool, \
             tc.tile_pool(name="ocpool", bufs=3) as ocpool, \
             tc.tile_pool(name="r2pool", bufs=2) as r2pool, \
             tc.tile_pool(name="opool", bufs=3) as opool:

            # ---- input DMAs ----
            # sync queue: w even tiles first, then small consts, xt chunk1,
            # wout.  gpsimd queue: w odd tiles, xt chunks 2-3.  scalar queue:
            # binary mask + xt chunk0 (then free for exps).
            w_sb = const.tile([128, NDT, 3 * GCOLS], BF16)
            xt_sb = const.tile([128, NDT, T], BF16)
            for a in range(NDT):
                eng = nc.sync if a % 2 == 0 else nc.gpsimd
                eng.dma_start(out=w_sb[:, a, :], in_=wqkv[a * 128 : (a + 1) * 128, :])

            for a in range(NDT):
                eng = nc.scalar if a < 6 else (nc.sync if a == 6 else nc.gpsimd)
                eng.dma_start(
                    out=xt_sb[:, a, 0:512], in_=xT[a * 128 : (a + 1) * 128, 0:512]
                )
            mask2_sb = const.tile([128, 2, 128], F32)
            nc.scalar.dma_start(
                out=mask2_sb, in_=mask2[:, :].rearrange("p (h c) -> p h c", c=128)
            )

            bqp_sb = const.tile([128, 4], F32)
            nc.sync.dma_start(out=bqp_sb, in_=bqp[:, :])
            ones1 = const.tile([1, 128], BF16)
            nc.sync.dma_start(out=ones1, in_=consts[0:1, :])
            bv_sb = const.tile([1, GCOLS], BF16)
            nc.sync.dma_start(out=bv_sb, in_=bv[:, :])
            for a in range(NDT):
                nc.sync.dma_start(
                    out=xt_sb[:, a, 512:1024], in_=xT[a * 128 : (a + 1) * 128, 512:1024]
                )
            wout_sb = const.tile([128, 2, D], BF16)
            for a in range(2):
                nc.sync.dma_start(
                    out=wout_sb[:, a, :], in_=wout[a * 128 : (a + 1) * 128, :]
                )
            for tch in range(2, NQC):
                for a in range(NDT):
                    nc.gpsimd.dma_start(
                        out=xt_sb[:, a, tch * 512 : (tch + 1) * 512],
                        in_=xT[a * 128 : (a + 1) * 128, tch * 512 : (tch + 1) * 512],
                    )

            # ---- persistent SBUF tensors ----
            qt = [const.tile([128, T], BF16, name=f"qt{p}") for p in range(2)]
            kt = [const.tile([128, T], BF16, name=f"kt{p}") for p in range(2)]
            v_aug = const.tile([128, NKT, HG * 65], BF16)
            ot = [const.tile([128, T], BF16, name=f"ot{p}") for p in range(2)]

            # ones column of v_aug (softmax denominators via the av matmul)
            ones64 = const.tile([128, NKT * HG], BF16)
            nc.vector.memset(ones64, 1.0)
            nc.vector.tensor_copy(
                v_aug.rearrange("p k (h c) -> p (k h) c", c=65)[:, :, 64], ones64
            )

            bvb_sb = const.tile([128, GCOLS], F32)
            bvb3 = bvb_sb.rearrange("p (h c) -> p h c", c=64)
            vview = v_aug.rearrange("p k (h c) -> p k h c", c=65)

            # ---- emission helpers ----
            fillers = collections.deque()

            def pump(n=1):
                for _ in range(n):
                    if fillers:
                        fillers.popleft()()

            def qkv_group_qt(qc, jt):
                # one [128,512] tile of qT (jt 0/1) or kT (jt 2/3), emitted as
                # two pump units (4 accumulation steps each) for finer filler
                # spreading
                state = {}

                def first():
                    qs = slice(qc * 512, (qc + 1) * 512)
                    ps = ps_qkv.tile([128, 512], F32, tag="qkv", name=f"qk_{qc}_{jt}")
                    state["ps"] = ps
                    for a in range(4):
                        nc.tensor.matmul(
                            ps,
                            w_sb[:, a, jt * 128 : (jt + 1) * 128],
                            xt_sb[:, a, qs],
                            start=(a == 0),
                            stop=False,
                        )

                def second():
                    qs = slice(qc * 512, (qc + 1) * 512)
                    ps = state["ps"]
                    for a in range(4, NDT):
                        nc.tensor.matmul(
                            ps,
                            w_sb[:, a, jt * 128 : (jt + 1) * 128],
                            xt_sb[:, a, qs],
                            start=False,
                            stop=(a == NDT - 1),
                        )
                    dst = qt[jt] if jt < 2 else kt[jt - 2]
                    if qc in (1, 2):
                        nc.scalar.activation(
                            dst[:, qs], ps, Ident, bias=bqp_sb[:, jt : jt + 1]
                        )
                    else:
                        nc.vector.tensor_scalar_add(
                            dst[:, qs], ps, bqp_sb[:, jt : jt + 1]
                        )

                return first, second

            def qkv_group_v(k):
                # V rows for k-tile k: [128 tok, 256 dims] + bias, two pump units
                state = {}

                def first():
                    ps = ps_qkv.tile([128, 512], F32, tag="qkv", name=f"v_{k}")
                    state["ps"] = ps
                    for a in range(4):
                        nc.tensor.matmul(
                            ps[:, 0:GCOLS],
                            xt_sb[:, a, k * 128 : (k + 1) * 128],
                            w_sb[:, a, 2 * GCOLS : 3 * GCOLS],
                            start=(a == 0),
                            stop=False,
                        )

                def second():
                    ps = state["ps"]
                    for a in range(4, NDT):
                        nc.tensor.matmul(
                            ps[:, 0:GCOLS],
                            xt_sb[:, a, k * 128 : (k + 1) * 128],
                            w_sb[:, a, 2 * GCOLS : 3 * GCOLS],
                            start=False,
                            stop=(a == NDT - 1),
                        )
                    nc.vector.tensor_add(
                        vview[:, k, :, 0:64],
                        ps[:, 0:GCOLS].rearrange("p (h c) -> p h c", c=64),
                        bvb3,
                    )

                return first, second

            def proj_unit(tt, dc):
                def emit():
                    ps = ps_s.tile(
                        [128, 2, 512], F32, tag="s", name=f"pr_{tt}_{dc}"
                    )[:, dc, :]

                    nc.tensor.matmul(
                        ps,
                        ot[0][:, tt * 128 : (tt + 1) * 128],
                        wout_sb[:, 0, dc * 512 : (dc + 1) * 512],
                        start=True,
                        stop=False,
                    )
                    nc.tensor.matmul(
                        ps,
                        ot[1][:, tt * 128 : (tt + 1) * 128],
                        wout_sb[:, 1, dc * 512 : (dc + 1) * 512],
                        start=False,
                        stop=True,
                    )
                    o_sb = opool.tile([128, 512], BF16, tag="osb")
                    if tt >= 12 and dc == 0:
                        nc.scalar.activation(o_sb, ps, Ident)
                    else:
                        nc.vector.tensor_copy(o_sb, ps)
                    deng = (nc.sync, nc.gpsimd, nc.scalar)[(2 * tt + dc) % 3 if tt >= 12 else 0]
                    deng.dma_start(
                        out=out[tt * 128 : (tt + 1) * 128, dc * 512 : (dc + 1) * 512],
                        in_=o_sb,
                    )
                return emit

            def att(qc, pr):
                n_kt = 4 * qc + 4
                qs = slice(qc * 512, (qc + 1) * 512)
                o_ps = ps_o.tile([65, 2, 512], F32, tag="o", name=f"o_{qc}_{pr}")

                def emit_av(prev):
                    k, c0, pp = prev
                    cs = slice(c0, 512)
                    for hh in (0, 1):
                        hl = pr * 2 + hh
                        nc.tensor.matmul(
                            o_ps[:, hh, cs],
                            v_aug[:, k, hl * 65 : (hl + 1) * 65],
                            pp[:, hh, cs],
                            start=(k == 0),
                            stop=(k == n_kt - 1),
                        )

                # LAG-iteration software pipeline: av(k-LAG) runs while exp(k)
                # computes, so the PE never waits on the softmax path (HAM
                # micro-idle avoidance).  Both head-halves of a k-tile share
                # one 2-bank PSUM pair so the mask add and exp are single
                # strided instructions.
                LAG = 3
                pending = collections.deque()
                for k in range(n_kt + LAG):
                    if k < n_kt:
                        j = k - 4 * qc
                        c0 = j * 128 if j > 0 else 0
                        cs = slice(c0, 512)
                        s = ps_s.tile(
                            [128, 2, 512], F32, tag="s", name=f"s_{qc}_{pr}_{k}"
                        )
                        for hh in (0, 1):
                            half = slice(hh * 64, hh * 64 + 64)
                            nc.tensor.matmul(
                                s[:, hh, cs],
                                kt[pr][half, k * 128 : (k + 1) * 128],
                                qt[pr][half, qc * 512 + c0 : (qc + 1) * 512],
                                start=True,
                                stop=True,
                            )
                        if j >= 0:
                            blk = slice(c0, c0 + 128)
                            nc.vector.tensor_add(
                                s[:, :, blk], s[:, :, blk], mask2_sb
                            )
                        p = ppool.tile([128, 2, 512], BF16, tag="p")
                        nc.scalar.activation(p[:, :, cs], s[:, :, cs], Exp, scale=0.125)
                        pending.append((k, c0, p))
                    if len(pending) > LAG or k >= n_kt:
                        emit_av(pending.popleft())
                    pump(1)

                # evict o (incl. denominator row 64) to SBUF immediately —
                # one DVE hop frees both o PSUM banks so the next phase's av
                # matmuls never wait on the normalization chain below.
                rf0 = rpool.tile([1, 1024], F32, tag="rf0", name=f"rf0_{qc}_{pr}")
                nc.scalar.activation(
                    rf0.rearrange("p (h c) -> p h c", c=512), o_ps[64:65, :, :], Ident
                )
                oc = ocpool.tile([64, 2, 512], F32, tag="oc", name=f"oc_{qc}_{pr}")
                nc.scalar.activation(oc, o_ps[0:64, :, :], Ident)
                # normalization (all off the PE critical path): reciprocal of
                # denominators -> bf16 -> broadcast via two K=1 matmuls ->
                # per-column mul into ot
                rf = rpool.tile([1, 1024], F32, tag="rf", name=f"rf_{qc}_{pr}")
                nc.vector.reciprocal_approx_fast(out=rf, in_=rf0)
                rfb = rpool.tile([1, 1024], BF16, tag="rfb", name=f"rfb_{qc}_{pr}")
                nc.vector.tensor_copy(rfb, rf)
                r2_sb = r2pool.tile([64, 2, 512], BF16, tag="r2")
                nc.gpsimd.partition_broadcast(r2_sb[:, 0, :], rfb[0:1, 0:512])
                nc.gpsimd.partition_broadcast(r2_sb[:, 1, :], rfb[0:1, 512:1024])
                for hh in (0, 1):
                    half = slice(hh * 64, hh * 64 + 64)
                    nc.vector.tensor_mul(
                        ot[pr][half, qs], oc[:, hh, :], r2_sb[:, hh, :]
                    )

            # ---- HAM pre-warm: ~5us of dummy matmuls while input DMAs are
            # in flight, so the PE clock is at 8/8 when real work starts ----
            warm_sb = const.tile([128, 512], BF16, name="warm")
            nc.vector.memset(warm_sb, 0.0)
            for i in range(6):
                warm_ps = ps_qkv.tile([128, 512], F32, tag="qkv", name=f"warm{i}")
                for r in range(8):
                    nc.tensor.matmul(
                        warm_ps,
                        warm_sb[:, 0:128],
                        warm_sb,
                        start=(r == 0),
                        stop=(r == 7),
                    )

            # ---- main schedule ----
            # QKV(0) emitted directly; later QKV and all PROJ work pumped as
            # PE fillers during ATT, with proj deferred to the late (supply-
            # starved) phases so the PE never idles on the softmax path.
            for jt in range(4):
                f1, f2 = qkv_group_qt(0, jt)
                f1(); f2()
            # broadcast V-bias to all partitions: ones1.T @ bv -> [128, GCOLS]
            bvb_ps = ps_qkv.tile([128, 512], F32, tag="qkv", name="bvb")
            nc.tensor.matmul(bvb_ps[:, 0:GCOLS], ones1, bv_sb, start=True, stop=True)
            nc.vector.tensor_copy(bvb_sb, bvb_ps[:, 0:GCOLS])
            for k in range(4):
                f1, f2 = qkv_group_v(k)
                f1(); f2()

            def enq_qt(qc):
                for jt in range(4):
                    fillers.extend(qkv_group_qt(qc, jt))

            def enq_v(qc):
                for k in range(4 * qc, 4 * qc + 4):
                    fillers.extend(qkv_group_v(k))

            def enq_proj(qc):
                for tt in range(qc * 4, qc * 4 + 4):
                    for dc in range(2):
                        fillers.append(proj_unit(tt, dc))

            supply = {
                (0, 0): [lambda: enq_qt(1)],
                (0, 1): [lambda: enq_v(1)],
                (1, 0): [lambda: enq_qt(2)],
                (1, 1): [lambda: enq_v(2)],
                (2, 0): [lambda: enq_qt(3), lambda: enq_proj(0)],
                (2, 1): [lambda: enq_proj(1)],
                (3, 0): [lambda: enq_v(3), lambda: enq_proj(2)],
                (3, 1): [],
            }
            for qc in range(NQC):
                for pr in (0, 1):
                    for enq in supply[(qc, pr)]:
                        enq()
                    att(qc, pr)
            enq_proj(3)
            while fillers:
                pump(1)

    nc.compile()
    return nc


def _mask2_np():
    rk = np.arange(128)[:, None]
    cq = np.arange(128)[None, :]
    m = np.where(rk <= cq, 0.0, -1.0e30).astype(np.float32)
    return np.ascontiguousarray(np.concatenate([m, m], axis=1))


def _consts_np():
    c = np.zeros((3, 128), dtype=np.float32)
    c[0, :] = 1.0
    c[1, 0:64] = 1.0
    c[2, 64:128] = 1.0
    return c.astype(BF)


def _in_maps(x, w_qkv, b_qkv, w_out):
    mask2 = _mask2_np()
    consts = _consts_np()
    xTs = [np.ascontiguousarray(x[b].T.astype(BF)) for b in range(B)]
    in_maps = []
    for c in range(8):
        b, g = divmod(c, 4)
        cols = slice(g * GCOLS, (g + 1) * GCOLS)
        wq = np.concatenate(
            [w_qkv[:, cols], w_qkv[:, D:][:, cols], w_qkv[:, 2 * D :][:, cols]], axis=1
        )
        bq = np.concatenate([b_qkv[cols], b_qkv[D:][cols], b_qkv[2 * D :][cols]])
        in_maps.append(
            {
                "xT": xTs[b],
                "wqkv": np.ascontiguousarray(wq.astype(BF)),
                "bqp": np.ascontiguousarray(bq[: 2 * GCOLS].reshape(4, 128).T),
                "bv": np.ascontiguousarray(bq[2 * GCOLS :].reshape(1, GCOLS).astype(BF)),
                "wout": np.ascontiguousarray(w_out[cols, :].astype(BF)),
                "mask2": mask2,
                "consts": consts,
            }
        )
    return in_maps


def kernel(x, w_qkv, b_qkv, w_out, b_out):
    x = np.ascontiguousarray(np.asarray(x, dtype=np.float32))
    w_qkv = np.ascontiguousarray(np.asarray(w_qkv, dtype=np.float32))
    b_qkv = np.asarray(b_qkv, dtype=np.float32)
    w_out = np.ascontiguousarray(np.asarray(w_out, dtype=np.float32))
    b_out = np.asarray(b_out, dtype=np.float32)

    if "nc" not in _CACHED:
        _CACHED["nc"] = _build()
    nc = _CACHED["nc"]

    res = run_bass_kernel_spmd(nc, _in_maps(x, w_qkv, b_qkv, w_out), list(range(8)))
    out = np.zeros((B, T, D), dtype=np.float32)
    for c in range(8):
        out[c // 4] += res.results[c]["out"].astype(np.float32)
    out += b_out
    return out


# revision 37
# speedup vs baseline: 1.0156x; 1.0156x over previous
"""Multi-head causal attention (B=2, T=2048, D=1024, H=16) on 8 trn2 NeuronCores.

Sharding: data-parallel over batch (2) x tensor-parallel over heads (4 groups of
4 heads). Core c handles batch c//4, head group c%4. Each core computes its
heads' attention and a partial output projection; the host sums the 4 partials
per batch and adds b_out.

v2: all-bf16 matmul operands (host-side casts), software-pipelined attention
inner loop with QKV/proj matmuls pumped as PE fillers, slim normalization path,
gpsimd offload for mask adds / normalize muls / PSUM evictions, bf16 output.
"""

import collections
import sys

sys.path.insert(0, "/opt/trn_rl_repo")

import ml_dtypes
import numpy as np

import concourse.bass as bass
import concourse.tile as tile
from concourse import bacc, mybir
from concourse.bass_utils import run_bass_kernel_spmd

F32 = mybir.dt.float32
BF16 = mybir.dt.bfloat16
BF = ml_dtypes.bfloat16

B, T, D, H = 2, 2048, 1024, 16
DH = D // H            # 64
HG = 4                 # heads per core
GCOLS = HG * DH        # 256 columns of q/k/v per core
NKT = T // 128         # 16 k-tiles of 128
NQC = T // 512         # 4 q-chunks of 512
NDT = D // 128         # 8 d-tiles of 128 (contraction)

_CACHED = {}


def _build():
    nc = bacc.Bacc("TRN2", target_bir_lowering=False, debug=False, num_devices=8)

    xT = nc.dram_tensor("xT", [D, T], BF16, kind="ExternalInput").ap()
    wqkv = nc.dram_tensor("wqkv", [D, 3 * GCOLS], BF16, kind="ExternalInput").ap()
    bqp = nc.dram_tensor("bqp", [128, 4], F32, kind="ExternalInput").ap()
    bv = nc.dram_tensor("bv", [1, GCOLS], BF16, kind="ExternalInput").ap()
    wout = nc.dram_tensor("wout", [GCOLS, D], BF16, kind="ExternalInput").ap()
    # additive causal mask for diagonal blocks, duplicated for both halves:
    # [128 k, 2*128 q] with 0.0 where k<=q else -1e30
    mask2 = nc.dram_tensor("mask2", [128, 256], F32, kind="ExternalInput").ap()
    # consts row 0: ones
    consts = nc.dram_tensor("consts", [3, 128], BF16, kind="ExternalInput").ap()
    out = nc.dram_tensor("out", [T, D], BF16, kind="ExternalOutput").ap()

    Exp = mybir.ActivationFunctionType.Exp
    Ident = mybir.ActivationFunctionType.Identity

    with tile.TileContext(nc) as tc:
        with tc.tile_pool(name="const", bufs=1) as const, \
             tc.tile_pool(name="ps_qkv", bufs=2, space=bass.MemorySpace.PSUM) as ps_qkv, \
             tc.tile_pool(name="ps_s", bufs=2, space=bass.MemorySpace.PSUM) as ps_s, \
             tc.tile_pool(name="ps_o", bufs=1, space=bass.MemorySpace.PSUM) as ps_o, \
             tc.tile_pool(name="ppool", bufs=8) as ppool, \
             tc.tile_pool(name="rpool", bufs=2) as rpool, \
             tc.tile_pool(name="ocpool", bufs=3) as ocpool, \
             tc.tile_pool(name="r2pool", bufs=2) as r2pool, \
             tc.tile_pool(name="opool", bufs=3) as opool:

            # ---- input DMAs ----
            # sync queue: w even tiles first, then small consts, xt chunk1,
            # wout.  gpsimd queue: w odd tiles, xt chunks 2-3.  scalar queue:
            # binary mask + xt chunk0 (then free for exps).
            w_sb = const.tile([128, NDT, 3 * GCOLS], BF16)
            xt_sb = const.tile([128, NDT, T], BF16)
            wv = wqkv.rearrange("(z p) c -> p z c", p=128)
            xv = xT.rearrange("(z p) t -> p z t", p=128)

            # tiny consts first (bvb matmul inputs)
            ones1 = const.tile([1, 128], BF16)
            nc.sync.dma_start(out=ones1, in_=consts[0:1, :])
            bv_sb = const.tile([1, GCOLS], BF16)
            nc.sync.dma_start(out=bv_sb, in_=bv[:, :])
            bqp_sb = const.tile([128, 4], F32)
            nc.sync.dma_start(out=bqp_sb, in_=bqp[:, :])

            # V-projection inputs first: w v-columns + xt chunk-0 k-tiles, so
            # the first real matmul group is ready ~4us earlier
            nc.sync.dma_start(
                out=w_sb[:, :, 2 * GCOLS : 3 * GCOLS], in_=wv[:, :, 2 * GCOLS :]
            )
            for kk in range(4):
                nc.scalar.dma_start(
                    out=xt_sb[:, :, kk * 128 : (kk + 1) * 128],
                    in_=xv[:, :, kk * 128 : (kk + 1) * 128],
                )
            # q/k weight column-tiles, alternating queues
            for jt in range(4):
                eng = nc.gpsimd if jt % 2 == 0 else nc.sync
                eng.dma_start(
                    out=w_sb[:, :, jt * 128 : (jt + 1) * 128],
                    in_=wv[:, :, jt * 128 : (jt + 1) * 128],
                )
            mask2_sb = const.tile([128, 2, 128], F32)
            nc.scalar.dma_start(
                out=mask2_sb, in_=mask2[:, :].rearrange("p (h c) -> p h c", c=128)
            )
            for a in range(NDT):
                nc.sync.dma_start(
                    out=xt_sb[:, a, 512:1024], in_=xT[a * 128 : (a + 1) * 128, 512:1024]
                )
            wout_sb = const.tile([128, 2, D], BF16)
            for a in range(2):
                nc.sync.dma_start(
                    out=wout_sb[:, a, :], in_=wout[a * 128 : (a + 1) * 128, :]
                )
            for tch in range(2, NQC):
                for a in range(NDT):
                    nc.gpsimd.dma_start(
                        out=xt_sb[:, a, tch * 512 : (tch + 1) * 512],
                        in_=xT[a * 128 : (a + 1) * 128, tch * 512 : (tch + 1) * 512],
                    )

            # ---- persistent SBUF tensors ----
            qt = [const.tile([128, T], BF16, name=f"qt{p}") for p in range(2)]
            kt = [const.tile([128, T], BF16, name=f"kt{p}") for p in range(2)]
            v_aug = const.tile([128, NKT, HG * 65], BF16)
            ot = [const.tile([128, T], BF16, name=f"ot{p}") for p in range(2)]

            # ones column of v_aug (softmax denominators via the av matmul)
            ones64 = const.tile([128, NKT * HG], BF16)
            nc.vector.memset(ones64, 1.0)
            nc.vector.tensor_copy(
                v_aug.rearrange("p k (h c) -> p (k h) c", c=65)[:, :, 64], ones64
            )

            bvb_sb = const.tile([128, GCOLS], F32)
            bvb3 = bvb_sb.rearrange("p (h c) -> p h c", c=64)
            vview = v_aug.rearrange("p k (h c) -> p k h c", c=65)

            # ---- emission helpers ----
            fillers = collections.deque()

            def pump(n=1):
                for _ in range(n):
                    if fillers:
                        fillers.popleft()()

            def qkv_group_qt(qc, jt):
                # one [128,512] tile of qT (jt 0/1) or kT (jt 2/3), emitted as
                # two pump units (4 accumulation steps each) for finer filler
                # spreading
                state = {}

                def first():
                    qs = slice(qc * 512, (qc + 1) * 512)
                    ps = ps_qkv.tile([128, 512], F32, tag="qkv", name=f"qk_{qc}_{jt}")
                    state["ps"] = ps
                    for a in range(4):
                        nc.tensor.matmul(
                            ps,
                            w_sb[:, a, jt * 128 : (jt + 1) * 128],
                            xt_sb[:, a, qs],
                            start=(a == 0),
                            stop=False,
                        )

                def second():
                    qs = slice(qc * 512, (qc + 1) * 512)
                    ps = state["ps"]
                    for a in range(4, NDT):
                        nc.tensor.matmul(
                            ps,
                            w_sb[:, a, jt * 128 : (jt + 1) * 128],
                            xt_sb[:, a, qs],
                            start=False,
                            stop=(a == NDT - 1),
                        )
                    dst = qt[jt] if jt < 2 else kt[jt - 2]
                    if qc in (1, 2):
                        nc.scalar.activation(
                            dst[:, qs], ps, Ident, bias=bqp_sb[:, jt : jt + 1]
                        )
                    else:
                        nc.vector.tensor_scalar_add(
                            dst[:, qs], ps, bqp_sb[:, jt : jt + 1]
                        )

                return first, second

            def qkv_group_v(k):
                # V rows for k-tile k: [128 tok, 256 dims] + bias, two pump units
                state = {}

                def first():
                    ps = ps_qkv.tile([128, 512], F32, tag="qkv", name=f"v_{k}")
                    state["ps"] = ps
                    for a in range(4):
                        nc.tensor.matmul(
                            ps[:, 0:GCOLS],
                            xt_sb[:, a, k * 128 : (k + 1) * 128],
                            w_sb[:, a, 2 * GCOLS : 3 * GCOLS],
                            start=(a == 0),
                            stop=False,
                        )

                def second():
                    ps = state["ps"]
                    for a in range(4, NDT):
                        nc.tensor.matmul(
                            ps[:, 0:GCOLS],
                            xt_sb[:, a, k * 128 : (k + 1) * 128],
                            w_sb[:, a, 2 * GCOLS : 3 * GCOLS],
                            start=False,
                            stop=(a == NDT - 1),
                        )
                    nc.vector.tensor_add(
                        vview[:, k, :, 0:64],
                        ps[:, 0:GCOLS].rearrange("p (h c) -> p h c", c=64),
                        bvb3,
                    )

                return first, second

            def proj_unit(tt, dc):
                def emit():
                    ps = ps_s.tile(
                        [128, 2, 512], F32, tag="s", name=f"pr_{tt}_{dc}"
                    )[:, dc, :]

                    nc.tensor.matmul(
                        ps,
                        ot[0][:, tt * 128 : (tt + 1) * 128],
                        wout_sb[:, 0, dc * 512 : (dc + 1) * 512],
                        start=True,
                        stop=False,
                    )
                    nc.tensor.matmul(
                        ps,
                        ot[1][:, tt * 128 : (tt + 1) * 128],
                        wout_sb[:, 1, dc * 512 : (dc + 1) * 512],
                        start=False,
                        stop=True,
                    )
                    o_sb = opool.tile([128, 512], BF16, tag="osb")
                    if tt >= 12 and dc == 0:
                        nc.scalar.activation(o_sb, ps, Ident)
                    else:
                        nc.vector.tensor_copy(o_sb, ps)
                    deng = (nc.sync, nc.gpsimd, nc.scalar)[(2 * tt + dc) % 3 if tt >= 12 else 0]
                    deng.dma_start(
                        out=out[tt * 128 : (tt + 1) * 128, dc * 512 : (dc + 1) * 512],
                        in_=o_sb,
                    )
                return emit

            def att(qc, pr):
                n_kt = 4 * qc + 4
                qs = slice(qc * 512, (qc + 1) * 512)
                o_ps = ps_o.tile([65, 2, 512], F32, tag="o", name=f"o_{qc}_{pr}")

                def emit_av(prev):
                    k, c0, pp = prev
                    cs = slice(c0, 512)
                    for hh in (0, 1):
                        hl = pr * 2 + hh
                        nc.tensor.matmul(
                            o_ps[:, hh, cs],
                            v_aug[:, k, hl * 65 : (hl + 1) * 65],
                            pp[:, hh, cs],
                            start=(k == 0),
                            stop=(k == n_kt - 1),
                        )

                # LAG-iteration software pipeline: av(k-LAG) runs while exp(k)
                # computes, so the PE never waits on the softmax path (HAM
                # micro-idle avoidance).  Both head-halves of a k-tile share
                # one 2-bank PSUM pair so the mask add and exp are single
                # strided instructions.
                LAG = 3
                pending = collections.deque()
                for k in range(n_kt + LAG):
                    if k < n_kt:
                        j = k - 4 * qc
                        c0 = j * 128 if j > 0 else 0
                        cs = slice(c0, 512)
                        s = ps_s.tile(
                            [128, 2, 512], F32, tag="s", name=f"s_{qc}_{pr}_{k}"
                        )
                        for hh in (0, 1):
                            half = slice(hh * 64, hh * 64 + 64)
                            nc.tensor.matmul(
                                s[:, hh, cs],
                                kt[pr][half, k * 128 : (k + 1) * 128],
                                qt[pr][half, qc * 512 + c0 : (qc + 1) * 512],
                                start=True,
                                stop=True,
                            )
                        if j >= 0:
                            blk = slice(c0, c0 + 128)
                            nc.vector.tensor_add(
                                s[:, :, blk], s[:, :, blk], mask2_sb
                            )
                        p = ppool.tile([128, 2, 512], BF16, tag="p")
                        nc.scalar.activation(p[:, :, cs], s[:, :, cs], Exp, scale=0.125)
                        pending.append((k, c0, p))
                    if len(pending) > LAG or k >= n_kt:
                        emit_av(pending.popleft())
                    pump(1)

                # evict o (incl. denominator row 64) to SBUF immediately —
                # one DVE hop frees both o PSUM banks so the next phase's av
                # matmuls never wait on the normalization chain below.
                rf0 = rpool.tile([1, 1024], F32, tag="rf0", name=f"rf0_{qc}_{pr}")
                nc.vector.tensor_copy(
                    rf0.rearrange("p (h c) -> p h c", c=512), o_ps[64:65, :, :]
                )
                oc = ocpool.tile([64, 2, 512], F32, tag="oc", name=f"oc_{qc}_{pr}")
                nc.vector.tensor_copy(oc, o_ps[0:64, :, :])
                # normalization (all off the PE critical path): reciprocal of
                # denominators -> bf16 -> broadcast via two K=1 matmuls ->
                # per-column mul into ot
                rf = rpool.tile([1, 1024], F32, tag="rf", name=f"rf_{qc}_{pr}")
                nc.vector.reciprocal_approx_fast(out=rf, in_=rf0)
                rfb = rpool.tile([1, 1024], BF16, tag="rfb", name=f"rfb_{qc}_{pr}")
                nc.vector.tensor_copy(rfb, rf)
                r2_sb = r2pool.tile([64, 2, 512], BF16, tag="r2")
                nc.gpsimd.partition_broadcast(r2_sb[:, 0, :], rfb[0:1, 0:512])
                nc.gpsimd.partition_broadcast(r2_sb[:, 1, :], rfb[0:1, 512:1024])
                for hh in (0, 1):
                    half = slice(hh * 64, hh * 64 + 64)
                    nc.vector.tensor_mul(
                        ot[pr][half, qs], oc[:, hh, :], r2_sb[:, hh, :]
                    )

            # ---- HAM pre-warm: ~5us of dummy matmuls while input DMAs are
            # in flight, so the PE clock is at 8/8 when real work starts ----
            warm_sb = const.tile([128, 512], BF16, name="warm")
            nc.vector.memset(warm_sb, 0.0)
            for i in range(6):
                warm_ps = ps_qkv.tile([128, 512], F32, tag="qkv", name=f"warm{i}")
                for r in range(8):
                    nc.tensor.matmul(
                        warm_ps,
                        warm_sb[:, 0:128],
                        warm_sb,
                        start=(r == 0),
                        stop=(r == 7),
                    )

            # ---- main schedule ----
            # QKV(0) emitted directly (V first: its inputs land first); later
            # QKV and all PROJ work pumped as PE fillers during ATT, with proj
            # deferred to the late (supply-starved) phases so the PE never
            # idles on the softmax path.
            # broadcast V-bias to all partitions: ones1.T @ bv -> [128, GCOLS]
            bvb_ps = ps_qkv.tile([128, 512], F32, tag="qkv", name="bvb")
            nc.tensor.matmul(bvb_ps[:, 0:GCOLS], ones1, bv_sb, start=True, stop=True)
            nc.vector.tensor_copy(bvb_sb, bvb_ps[:, 0:GCOLS])
            for k in range(4):
                f1, f2 = qkv_group_v(k)
                f1(); f2()
            for jt in range(4):
                f1, f2 = qkv_group_qt(0, jt)
                f1(); f2()

            def enq_qt(qc):
                for jt in range(4):
                    fillers.extend(qkv_group_qt(qc, jt))

            def enq_v(qc):
                for k in range(4 * qc, 4 * qc + 4):
                    fillers.extend(qkv_group_v(k))

            def enq_proj(qc):
                for tt in range(qc * 4, qc * 4 + 4):
                    for dc in range(2):
                        fillers.append(proj_unit(tt, dc))

            supply = {
                (0, 0): [lambda: enq_qt(1)],
                (0, 1): [lambda: enq_v(1)],
                (1, 0): [lambda: enq_qt(2)],
                (1, 1): [lambda: enq_v(2)],
                (2, 0): [lambda: enq_qt(3), lambda: enq_proj(0)],
                (2, 1): [lambda: enq_proj(1)],
                (3, 0): [lambda: enq_v(3), lambda: enq_proj(2)],
                (3, 1): [],
            }
            for qc in range(NQC):
                for pr in (0, 1):
                    for enq in supply[(qc, pr)]:
                        enq()
                    att(qc, pr)
            enq_proj(3)
            while fillers:
                pump(1)

    nc.compile()
    return nc


def _mask2_np():
    rk = np.arange(128)[:, None]
    cq = np.arange(128)[None, :]
    m = np.where(rk <= cq, 0.0, -1.0e30).astype(np.float32)
    return np.ascontiguousarray(np.concatenate([m, m], axis=1))


def _consts_np():
    c = np.zeros((3, 128), dtype=np.float32)
    c[0, :] = 1.0
    c[1, 0:64] = 1.0
    c[2, 64:128] = 1.0
    return c.astype(BF)


def _in_maps(x, w_qkv, b_qkv, w_out):
    mask2 = _mask2_np()
    consts = _consts_np()
    xTs = [np.ascontiguousarray(x[b].T.astype(BF)) for b in range(B)]
    in_maps = []
    for c in range(8):
        b, g = divmod(c, 4)
        cols = slice(g * GCOLS, (g + 1) * GCOLS)
        wq = np.concatenate(
            [w_qkv[:, cols], w_qkv[:, D:][:, cols], w_qkv[:, 2 * D :][:, cols]], axis=1
        )
        bq = np.concatenate([b_qkv[cols], b_qkv[D:][cols], b_qkv[2 * D :][cols]])
        in_maps.append(
            {
                "xT": xTs[b],
                "wqkv": np.ascontiguousarray(wq.astype(BF)),
                "bqp": np.ascontiguousarray(bq[: 2 * GCOLS].reshape(4, 128).T),
                "bv": np.ascontiguousarray(bq[2 * GCOLS :].reshape(1, GCOLS).astype(BF)),
                "wout": np.ascontiguousarray(w_out[cols, :].astype(BF)),
                "mask2": mask2,
                "consts": consts,
            }
        )
    return in_maps


def kernel(x, w_qkv, b_qkv, w_out, b_out):
    x = np.ascontiguousarray(np.asarray(x, dtype=np.float32))
    w_qkv = np.ascontiguousarray(np.asarray(w_qkv, dtype=np.float32))
    b_qkv = np.asarray(b_qkv, dtype=np.float32)
    w_out = np.ascontiguousarray(np.asarray(w_out, dtype=np.float32))
    b_out = np.asarray(b_out, dtype=np.float32)

    if "nc" not in _CACHED:
        _CACHED["nc"] = _build()
    nc = _CACHED["nc"]

    res = run_bass_kernel_spmd(nc, _in_maps(x, w_qkv, b_qkv, w_out), list(range(8)))
    out = np.zeros((B, T, D), dtype=np.float32)
    for c in range(8):
        out[c // 4] += res.results[c]["out"].astype(np.float32)
    out += b_out
    return out
